# revision 4
# baseline (speedup 1.0000x reference)
"""Trainium2 Bass kernel for the cross-batch retrieval contrastive loss.

Pipeline per batch b (reference semantics):
  sent_mean = mean(sent_feat * masks)                     (host)
  v1   = conv1([bef^T; broadcast sent_mean])              -> (196, 512)
  MHA over 196 positions, out_proj                        -> (196, 512)
  mod  = conv2(o)                                         -> (196, 512)
  ql   = mod @ q_w^T + q_b ; kl = aft @ k_w^T + k_b       -> (196, 512)
  logits[a,b,l,m] = ql[a,l,:] . kl[b,m,:]   (head split is a no-op)
  t2v[a,b] = mean_l max_m ; v2t[a,b] = mean_m max_l
  loss = symmetric InfoNCE on S = 0.5*(t2v+v2t)*exp(logit_scale)  (host, 32x32)

Sharding: data-parallel over the query-batch axis 'a' (4 batches/core x 8
cores). Each core computes kl for all 32 key batches (replicated), its own
front-end, and both orientations of every (a, b) logits tile so that the
max over l and the max over m are both free-axis DVE reductions.

All matmuls run in bf16 with fp32 PSUM accumulation; the (32,32) final
cross-entropy runs on host in float64. Everything is feature-major
(d on partitions, positions on the free axis) except v (position-major,
needed as the stationary operand of attn @ v).
"""

import numpy as np
import ml_dtypes

B, LV, LT, D, H = 32, 196, 40, 512, 8
NCORES = 8
AL = B // NCORES          # query batches per core
KT = D // 128             # 128-row feature tiles per 512-dim tensor
LSPLIT = [(0, 128), (128, 68)]   # 196 = 128 + 68
BF16 = ml_dtypes.bfloat16

WNAMES = ["w1aT", "wqT", "wkT", "wvT", "woT", "wc2T", "wqlT", "wklT"]
BNAMES = ["bc2", "bql", "bkl"]

_CACHE = {}


def _build_program():
    from contextlib import ExitStack
    import concourse.bass as bass
    import concourse.bacc as bacc
    import concourse.tile as tile
    from concourse import mybir

    f32 = mybir.dt.float32
    bf = mybir.dt.bfloat16
    AX = mybir.AxisListType.X
    MAX = mybir.AluOpType.max
    EXP = mybir.ActivationFunctionType.Exp

    nc = bacc.Bacc("TRN2", target_bir_lowering=False, debug=False,
                   num_devices=NCORES)

    d_befT = nc.dram_tensor("befT", [D, AL * LV], bf, kind="ExternalInput").ap()
    d_aftT = nc.dram_tensor("aftT", [D, B * LV], bf, kind="ExternalInput").ap()
    d_txtc = nc.dram_tensor("txtc", [1, AL * D], bf, kind="ExternalInput").ap()
    d_w = {n: nc.dram_tensor(n, [D, D], bf, kind="ExternalInput").ap()
           for n in WNAMES}
    d_b = {n: nc.dram_tensor(n, [1, D], bf, kind="ExternalInput").ap()
           for n in BNAMES}
    d_out = nc.dram_tensor("out", [1, 2 * AL * B], f32, kind="ExternalOutput").ap()

    with tile.TileContext(nc) as tc, ExitStack() as ctx:
        const = ctx.enter_context(tc.tile_pool(name="const", bufs=1))
        big = ctx.enter_context(tc.tile_pool(name="big", bufs=1))
        fe = ctx.enter_context(tc.tile_pool(name="fe", bufs=2))
        ps = ctx.enter_context(tc.tile_pool(name="ps", bufs=6, space="PSUM"))
        ps2 = ctx.enter_context(tc.tile_pool(name="ps2", bufs=2, space="PSUM"))

        # ---- constants / weights into SBUF ----
        ones = const.tile([128, 256], bf, name="ones", tag="ones")
        nc.vector.memset(ones[:], 1.0)

        txtc = const.tile([1, AL * D], bf, name="txtc_sb", tag="txtc_sb")
        nc.sync.dma_start(out=txtc[:], in_=d_txtc[:, :])
        bias = {}
        for n in BNAMES:
            bias[n] = const.tile([1, D], bf, name=f"{n}_sb", tag=f"{n}_sb")
            nc.sync.dma_start(out=bias[n][:], in_=d_b[n][:, :])
        w = {}
        for n in WNAMES:
            w[n] = []
            for k in range(KT):
                t = const.tile([128, D], bf, name=f"{n}_{k}", tag=f"{n}_{k}")
                nc.sync.dma_start(out=t[:], in_=d_w[n][k * 128:(k + 1) * 128, :])
                w[n].append(t)

        aft = []
        for k in range(KT):
            t = big.tile([128, B * LV], bf, name=f"aft_{k}", tag=f"aft_{k}")
            nc.sync.dma_start(out=t[:], in_=d_aftT[k * 128:(k + 1) * 128, :])
            aft.append(t)
        befT = []
        for k in range(KT):
            t = big.tile([128, AL * LV], bf, name=f"bef_{k}", tag=f"bef_{k}")
            nc.sync.dma_start(out=t[:], in_=d_befT[k * 128:(k + 1) * 128, :])
            befT.append(t)
        klT = [big.tile([128, B * LV], bf, name=f"klT_{k}", tag=f"klT_{k}")
               for k in range(KT)]
        qlT = [big.tile([128, AL * LV], bf, name=f"qlT_{k}", tag=f"qlT_{k}")
               for k in range(KT)]
        out_sb = const.tile([1, 2 * AL * B], f32, name="out_sb", tag="out_sb")

        def proj(dst_tiles, dst_col, src_tiles, src_col, wname, bname=None,
                 n=LV):
            """dst[m][:, dst_col:dst_col+n] (bf16) =
               W^T-tiles x src[k][:, src_col:src_col+n] (+ bias rank-1)."""
            for m in range(KT):
                p = ps.tile([128, n], f32, name="p_proj", tag="ps")
                ms = slice(m * 128, (m + 1) * 128)
                for k in range(KT):
                    nc.tensor.matmul(p[:], lhsT=w[wname][k][:, ms],
                                     rhs=src_tiles[k][:, src_col:src_col + n],
                                     start=(k == 0), stop=(k == KT - 1 and bname is None))
                if bname is not None:
                    nc.tensor.matmul(p[:], lhsT=bias[bname][0:1, ms],
                                     rhs=ones[0:1, 0:n], start=False, stop=True)
                nc.vector.tensor_copy(dst_tiles[m][:, dst_col:dst_col + n], p[:])

        # ---- phase 1: klT for all 32 key batches ----
        for b in range(B):
            proj(klT, b * LV, aft, b * LV, "wklT", "bkl")

        # ---- phase 2: front-end for the 4 local query batches ----
        for a in range(AL):
            cs = a * LV
            v1 = [fe.tile([128, LV], bf, name=f"v1_{m}", tag=f"v1_{m}")
                  for m in range(KT)]
            # conv1: W1a @ befT + rank-1 txt contribution (incl. conv1_b)
            for m in range(KT):
                p = ps.tile([128, LV], f32, name="p_c1", tag="ps")
                ms = slice(m * 128, (m + 1) * 128)
                for k in range(KT):
                    nc.tensor.matmul(p[:], lhsT=w["w1aT"][k][:, ms],
                                     rhs=befT[k][:, cs:cs + LV],
                                     start=(k == 0), stop=False)
                nc.tensor.matmul(p[:],
                                 lhsT=txtc[0:1, a * D + m * 128: a * D + (m + 1) * 128],
                                 rhs=ones[0:1, 0:LV], start=False, stop=True)
                nc.vector.tensor_copy(v1[m][:], p[:])

            qt = [fe.tile([128, LV], bf, name=f"qt_{m}", tag=f"qt_{m}")
                  for m in range(KT)]
            kt = [fe.tile([128, LV], bf, name=f"kt_{m}", tag=f"kt_{m}")
                  for m in range(KT)]
            proj(qt, 0, v1, 0, "wqT")
            proj(kt, 0, v1, 0, "wkT")

            # v position-major: (196, 512) as two row tiles
            vpos = []
            for lt, (l0, ln) in enumerate(LSPLIT):
                p5 = ps.tile([ln, D], f32, name="p_vpos", tag="ps")
                for k in range(KT):
                    nc.tensor.matmul(p5[:], lhsT=v1[k][:, l0:l0 + ln],
                                     rhs=w["wvT"][k][:, :],
                                     start=(k == 0), stop=(k == KT - 1))
                t = fe.tile([ln, D], bf, name=f"vpos_{lt}", tag=f"vpos_{lt}")
                nc.vector.tensor_copy(t[:], p5[:])
                vpos.append(t)

            # attention, two heads per 128-partition group
            ot = []
            for kt2 in range(KT):
                po = ps.tile([128, LV], f32, name="p_o", tag="ps")
                pzb = ps.tile([128, LV], f32, name="p_zb", tag="ps")
                for hh in range(2):
                    h = kt2 * 2 + hh
                    off = 64 * hh
                    eT = []
                    for mt, (m0, mn) in enumerate(LSPLIT):
                        psc = ps.tile([mn, LV], f32, name="p_sc", tag="ps")
                        nc.tensor.matmul(psc[:],
                                         lhsT=kt[kt2][off:off + 64, m0:m0 + mn],
                                         rhs=qt[kt2][off:off + 64, :],
                                         start=True, stop=True)
                        e = fe.tile([mn, LV], bf, name=f"eT_{mt}", tag=f"eT_{mt}")
                        nc.scalar.activation(e[:], psc[:], EXP, scale=0.125)
                        eT.append(e)
                    pz = ps2.tile([1, LV], f32, name="p_z", tag="ps2")
                    for mt, (m0, mn) in enumerate(LSPLIT):
                        nc.tensor.matmul(pz[:], lhsT=ones[0:mn, 0:1],
                                         rhs=eT[mt][:], start=(mt == 0),
                                         stop=(mt == 1))
                    rz32 = fe.tile([1, LV], f32, name="rz32", tag="rz32")
                    nc.vector.reciprocal(rz32[:], pz[:])
                    rzb = fe.tile([1, LV], bf, name="rzb", tag="rzb")
                    nc.vector.tensor_copy(rzb[:], rz32[:])
                    # broadcast 1/Z down this head's 64 partitions
                    nc.tensor.matmul(pzb[off:off + 64, :], lhsT=ones[0:1, 0:64],
                                     rhs=rzb[0:1, :], start=True, stop=True)
                    # unnormalized o^T for this head
                    for mt, (m0, mn) in enumerate(LSPLIT):
                        nc.tensor.matmul(po[off:off + 64, :],
                                         lhsT=vpos[mt][:, h * 64:(h + 1) * 64],
                                         rhs=eT[mt][:], start=(mt == 0),
                                         stop=(mt == 1))
                zb = fe.tile([128, LV], f32, name="zb", tag="zb")
                nc.vector.tensor_copy(zb[:], pzb[:])
                t = fe.tile([128, LV], bf, name=f"ot_{kt2}", tag=f"ot_{kt2}")
                nc.vector.tensor_mul(t[:], po[:], zb[:])
                ot.append(t)

            pt = [fe.tile([128, LV], bf, name=f"pt_{m}", tag=f"pt_{m}")
                  for m in range(KT)]
            ct = [fe.tile([128, LV], bf, name=f"ct_{m}", tag=f"ct_{m}")
                  for m in range(KT)]
            proj(pt, 0, ot, 0, "woT")
            proj(ct, 0, pt, 0, "wc2T", "bc2")
            proj(qlT, cs, ct, 0, "wqlT", "bql")

        # ---- phase 3/4: all-pairs logits, both orientations ----
        # orient 1: rows = l of a, free = m of b  -> max over m -> t2v
        # orient 2: rows = m of b, free = l of a  -> max over l -> v2t
        for orient in range(2):
            for a in range(AL):
                pacc = ps2.tile([1, B], f32, name="p_acc", tag="ps2")
                for lt, (l0, ln) in enumerate(LSPLIT):
                    rm = fe.tile([ln, B], bf, name=f"rm_{lt}", tag=f"rm_{lt}")
                    for bg in range(B // 4):
                        pts = [ps.tile([ln, LV], f32, name="p_lg", tag="ps")
                               for _ in range(4)]
                        for bi in range(4):
                            b = bg * 4 + bi
                            for k in range(KT):
                                if orient == 0:
                                    lhsT = qlT[k][:, a * LV + l0: a * LV + l0 + ln]
                                    rhs = klT[k][:, b * LV:(b + 1) * LV]
                                else:
                                    lhsT = klT[k][:, b * LV + l0: b * LV + l0 + ln]
                                    rhs = qlT[k][:, a * LV:(a + 1) * LV]
                                nc.tensor.matmul(pts[bi][:], lhsT=lhsT, rhs=rhs,
                                                 start=(k == 0), stop=(k == KT - 1))
                        for bi in range(4):
                            b = bg * 4 + bi
                            nc.vector.tensor_reduce(rm[0:ln, b:b + 1], pts[bi][:],
                                                    axis=AX, op=MAX)
                    nc.tensor.matmul(pacc[:], lhsT=ones[0:ln, 0:1], rhs=rm[:],
                                     start=(lt == 0), stop=(lt == 1))
                col = orient * AL * B + a * B
                nc.scalar.mul(out_sb[0:1, col:col + B], pacc[:], 1.0 / LV)

        nc.sync.dma_start(out=d_out[:, :], in_=out_sb[:])

    nc.compile()
    return nc


def get_program():
    if "nc" not in _CACHE:
        _CACHE["nc"] = _build_program()
    return _CACHE["nc"]


def make_in_maps(bef_feat, sent_feat, aft_feat, masks,
                 conv1_w, conv1_b, in_proj_w, out_proj_w, conv2_w, conv2_b,
                 q_w, q_b, k_w, k_b, logit_scale):
    bef_feat = np.asarray(bef_feat, np.float32)
    sent_feat = np.asarray(sent_feat, np.float32)
    aft_feat = np.asarray(aft_feat, np.float32)
    masks = np.asarray(masks, np.float32)

    sent_mean = (sent_feat * masks[:, :, None]).mean(axis=1)       # (B, D)
    txtc = sent_mean @ np.asarray(conv1_w, np.float32)[:, D:].T \
        + np.asarray(conv1_b, np.float32)                          # (B, D)

    aftT = np.ascontiguousarray(
        aft_feat.transpose(2, 0, 1).reshape(D, B * LV)).astype(BF16)

    wmats = {
        "w1aT": np.asarray(conv1_w, np.float32)[:, :D].T,
        "wqT": np.asarray(in_proj_w, np.float32)[0:D, :].T,
        "wkT": np.asarray(in_proj_w, np.float32)[D:2 * D, :].T,
        "wvT": np.asarray(in_proj_w, np.float32)[2 * D:3 * D, :].T,
        "woT": np.asarray(out_proj_w, np.float32).T,
        "wc2T": np.asarray(conv2_w, np.float32).T,
        "wqlT": np.asarray(q_w, np.float32).T,
        "wklT": np.asarray(k_w, np.float32).T,
    }
    wmats = {n: np.ascontiguousarray(v).astype(BF16) for n, v in wmats.items()}
    bvecs = {
        "bc2": np.asarray(conv2_b, np.float32).reshape(1, D).astype(BF16),
        "bql": np.asarray(q_b, np.float32).reshape(1, D).astype(BF16),
        "bkl": np.asarray(k_b, np.float32).reshape(1, D).astype(BF16),
    }

    in_maps = []
    for c in range(NCORES):
        sl = slice(c * AL, (c + 1) * AL)
        befT = np.ascontiguousarray(
            bef_feat[sl].transpose(2, 0, 1).reshape(D, AL * LV)).astype(BF16)
        m = {"befT": befT, "aftT": aftT,
             "txtc": np.ascontiguousarray(
                 txtc[sl].reshape(1, AL * D)).astype(BF16)}
        m.update(wmats)
        m.update(bvecs)
        in_maps.append(m)
    return in_maps


def finish(outs, logit_scale):
    """outs: list of 8 per-core (1, 256) arrays -> scalar loss."""
    t2v = np.zeros((B, B), np.float64)
    v2t = np.zeros((B, B), np.float64)
    for c in range(NCORES):
        o = np.asarray(outs[c], np.float64).reshape(-1)
        for a in range(AL):
            t2v[c * AL + a, :] = o[a * B:(a + 1) * B]
            v2t[c * AL + a, :] = o[AL * B + a * B: AL * B + (a + 1) * B]
    S = 0.5 * (t2v + v2t) * np.exp(np.float64(np.asarray(logit_scale)))

    def ce(m):
        lse = np.log(np.sum(np.exp(m - m.max(axis=1, keepdims=True)), axis=1)) \
            + m.max(axis=1)
        return -np.mean(np.diag(m) - lse)

    loss = 0.5 * (ce(S) + ce(S.T))
    return np.float32(loss)


def kernel(**inputs):
    from concourse.bass_utils import run_bass_kernel_spmd

    nc = get_program()
    in_maps = make_in_maps(**inputs)
    res = run_bass_kernel_spmd(nc, in_maps, core_ids=list(range(NCORES)))
    outs = [res.results[c]["out"] for c in range(NCORES)]
    return finish(outs, inputs["logit_scale"])


# revision 7
# speedup vs baseline: 71.2023x; 71.2023x over previous
"""Trainium2 Bass kernel for the cross-batch retrieval contrastive loss.

Pipeline per batch b (reference semantics):
  sent_mean = mean(sent_feat * masks)                     (host)
  v1   = conv1([bef^T; broadcast sent_mean])              -> (196, 512)
  MHA over 196 positions, out_proj                        -> (196, 512)
  mod  = conv2(o)                                         -> (196, 512)
  ql   = mod @ q_w^T + q_b ; kl = aft @ k_w^T + k_b       -> (196, 512)
  logits[a,b,l,m] = ql[a,l,:] . kl[b,m,:]   (head split is a no-op)
  t2v[a,b] = mean_l max_m ; v2t[a,b] = mean_m max_l
  loss = symmetric InfoNCE on S = 0.5*(t2v+v2t)*exp(logit_scale)  (host, 32x32)

Sharding: data-parallel over the query-batch axis 'a' (4 batches/core x 8
cores). Each core computes kl for all 32 key batches (replicated), its own
front-end, and both orientations of every (a, b) logits tile so that the
max over l and the max over m are both free-axis DVE reductions.

All matmuls run in bf16 with fp32 PSUM accumulation; the (32,32) final
cross-entropy runs on host in float64. Everything is feature-major
(d on partitions, positions on the free axis) except v (position-major,
needed as the stationary operand of attn @ v).
"""

import numpy as np
import ml_dtypes

B, LV, LT, D, H = 32, 196, 40, 512, 8
NCORES = 8
AL = B // NCORES          # query batches per core
KT = D // 128             # 128-row feature tiles per 512-dim tensor
LSPLIT = [(0, 128), (128, 68)]   # 196 = 128 + 68
BF16 = ml_dtypes.bfloat16

WNAMES = ["w1aT", "wqT", "wkT", "wvT", "woT", "wc2T", "wqlT", "wklT"]
BNAMES = ["bc2", "bql", "bkl"]

_CACHE = {}


def _build_program(reps=1):
    from contextlib import ExitStack
    import concourse.bass as bass
    import concourse.bacc as bacc
    import concourse.tile as tile
    from concourse import mybir

    f32 = mybir.dt.float32
    bf = mybir.dt.bfloat16
    AX = mybir.AxisListType.X
    MAX = mybir.AluOpType.max
    EXP = mybir.ActivationFunctionType.Exp

    nc = bacc.Bacc("TRN2", target_bir_lowering=False, debug=False,
                   num_devices=NCORES)

    d_befT = nc.dram_tensor("befT", [D, AL * LV], bf, kind="ExternalInput").ap()
    d_aftT = nc.dram_tensor("aftT", [D, B * LV], bf, kind="ExternalInput").ap()
    d_txtc = nc.dram_tensor("txtc", [1, AL * D], bf, kind="ExternalInput").ap()
    d_w = {n: nc.dram_tensor(n, [D, D], bf, kind="ExternalInput").ap()
           for n in WNAMES}
    d_b = {n: nc.dram_tensor(n, [1, D], bf, kind="ExternalInput").ap()
           for n in BNAMES}
    d_out = nc.dram_tensor("out", [1, 2 * AL * B], f32, kind="ExternalOutput").ap()

    with tile.TileContext(nc) as tc, ExitStack() as ctx:
        const = ctx.enter_context(tc.tile_pool(name="const", bufs=1))
        big = ctx.enter_context(tc.tile_pool(name="big", bufs=1))
        fe = ctx.enter_context(tc.tile_pool(name="fe", bufs=2))
        ps = ctx.enter_context(tc.tile_pool(name="ps", bufs=6, space="PSUM"))
        ps2 = ctx.enter_context(tc.tile_pool(name="ps2", bufs=2, space="PSUM"))

        # ---- constants / weights into SBUF ----
        ones = const.tile([128, 256], bf, name="ones", tag="ones")
        nc.vector.memset(ones[:], 1.0)

        for _rep in range(reps):
            _kernel_body(nc, tc, ctx, mybir, const, big, fe, ps, ps2, ones,
                         d_befT, d_aftT, d_txtc, d_w, d_b, d_out)

    nc.compile()
    return nc


def _kernel_body(nc, tc, ctx, mybir, const, big, fe, ps, ps2, ones,
                 d_befT, d_aftT, d_txtc, d_w, d_b, d_out):
    f32 = mybir.dt.float32
    bf = mybir.dt.bfloat16
    AX = mybir.AxisListType.X
    MAX = mybir.AluOpType.max
    EXP = mybir.ActivationFunctionType.Exp

    if True:
        txtc = const.tile([1, AL * D], bf, name="txtc_sb", tag="txtc_sb")
        nc.sync.dma_start(out=txtc[:], in_=d_txtc[:, :])
        bias = {}
        for n in BNAMES:
            bias[n] = const.tile([1, D], bf, name=f"{n}_sb", tag=f"{n}_sb")
            nc.sync.dma_start(out=bias[n][:], in_=d_b[n][:, :])
        w = {}
        for n in WNAMES:
            w[n] = []
            for k in range(KT):
                t = const.tile([128, D], bf, name=f"{n}_{k}", tag=f"{n}_{k}")
                nc.sync.dma_start(out=t[:], in_=d_w[n][k * 128:(k + 1) * 128, :])
                w[n].append(t)

        aft = []
        for k in range(KT):
            t = big.tile([128, B * LV], bf, name=f"aft_{k}", tag=f"aft_{k}")
            nc.sync.dma_start(out=t[:], in_=d_aftT[k * 128:(k + 1) * 128, :])
            aft.append(t)
        befT = []
        for k in range(KT):
            t = big.tile([128, AL * LV], bf, name=f"bef_{k}", tag=f"bef_{k}")
            nc.sync.dma_start(out=t[:], in_=d_befT[k * 128:(k + 1) * 128, :])
            befT.append(t)
        klT = [big.tile([128, B * LV], bf, name=f"klT_{k}", tag=f"klT_{k}")
               for k in range(KT)]
        qlT = [big.tile([128, AL * LV], bf, name=f"qlT_{k}", tag=f"qlT_{k}")
               for k in range(KT)]
        out_sb = const.tile([1, 2 * AL * B], f32, name="out_sb", tag="out_sb")

        def proj(dst_tiles, dst_col, src_tiles, src_col, wname, bname=None,
                 n=LV):
            """dst[m][:, dst_col:dst_col+n] (bf16) =
               W^T-tiles x src[k][:, src_col:src_col+n] (+ bias rank-1)."""
            for m in range(KT):
                p = ps.tile([128, n], f32, name="p_proj", tag="ps")
                ms = slice(m * 128, (m + 1) * 128)
                for k in range(KT):
                    nc.tensor.matmul(p[:], lhsT=w[wname][k][:, ms],
                                     rhs=src_tiles[k][:, src_col:src_col + n],
                                     start=(k == 0), stop=(k == KT - 1 and bname is None))
                if bname is not None:
                    nc.tensor.matmul(p[:], lhsT=bias[bname][0:1, ms],
                                     rhs=ones[0:1, 0:n], start=False, stop=True)
                nc.vector.tensor_copy(dst_tiles[m][:, dst_col:dst_col + n], p[:])

        # ---- phase 1: klT for all 32 key batches ----
        for b in range(B):
            proj(klT, b * LV, aft, b * LV, "wklT", "bkl")

        # ---- phase 2: front-end for the 4 local query batches ----
        for a in range(AL):
            cs = a * LV
            v1 = [fe.tile([128, LV], bf, name=f"v1_{m}", tag=f"v1_{m}")
                  for m in range(KT)]
            # conv1: W1a @ befT + rank-1 txt contribution (incl. conv1_b)
            for m in range(KT):
                p = ps.tile([128, LV], f32, name="p_c1", tag="ps")
                ms = slice(m * 128, (m + 1) * 128)
                for k in range(KT):
                    nc.tensor.matmul(p[:], lhsT=w["w1aT"][k][:, ms],
                                     rhs=befT[k][:, cs:cs + LV],
                                     start=(k == 0), stop=False)
                nc.tensor.matmul(p[:],
                                 lhsT=txtc[0:1, a * D + m * 128: a * D + (m + 1) * 128],
                                 rhs=ones[0:1, 0:LV], start=False, stop=True)
                nc.vector.tensor_copy(v1[m][:], p[:])

            qt = [fe.tile([128, LV], bf, name=f"qt_{m}", tag=f"qt_{m}")
                  for m in range(KT)]
            kt = [fe.tile([128, LV], bf, name=f"kt_{m}", tag=f"kt_{m}")
                  for m in range(KT)]
            proj(qt, 0, v1, 0, "wqT")
            proj(kt, 0, v1, 0, "wkT")

            # v position-major: (196, 512) as two row tiles
            vpos = []
            for lt, (l0, ln) in enumerate(LSPLIT):
                p5 = ps.tile([ln, D], f32, name="p_vpos", tag="ps")
                for k in range(KT):
                    nc.tensor.matmul(p5[:], lhsT=v1[k][:, l0:l0 + ln],
                                     rhs=w["wvT"][k][:, :],
                                     start=(k == 0), stop=(k == KT - 1))
                t = fe.tile([ln, D], bf, name=f"vpos_{lt}", tag=f"vpos_{lt}")
                nc.vector.tensor_copy(t[:], p5[:])
                vpos.append(t)

            # attention, two heads per 128-partition group
            ot = []
            for kt2 in range(KT):
                po = ps.tile([128, LV], f32, name="p_o", tag="ps")
                pzb = ps.tile([128, LV], f32, name="p_zb", tag="ps")
                for hh in range(2):
                    h = kt2 * 2 + hh
                    off = 64 * hh
                    eT = []
                    for mt, (m0, mn) in enumerate(LSPLIT):
                        psc = ps.tile([mn, LV], f32, name="p_sc", tag="ps")
                        nc.tensor.matmul(psc[:],
                                         lhsT=kt[kt2][off:off + 64, m0:m0 + mn],
                                         rhs=qt[kt2][off:off + 64, :],
                                         start=True, stop=True)
                        e = fe.tile([mn, LV], bf, name=f"eT_{mt}", tag=f"eT_{mt}")
                        nc.scalar.activation(e[:], psc[:], EXP, scale=0.125)
                        eT.append(e)
                    pz = ps2.tile([1, LV], f32, name="p_z", tag="ps2")
                    for mt, (m0, mn) in enumerate(LSPLIT):
                        nc.tensor.matmul(pz[:], lhsT=ones[0:mn, 0:1],
                                         rhs=eT[mt][:], start=(mt == 0),
                                         stop=(mt == 1))
                    rz32 = fe.tile([1, LV], f32, name="rz32", tag="rz32")
                    nc.vector.reciprocal(rz32[:], pz[:])
                    rzb = fe.tile([1, LV], bf, name="rzb", tag="rzb")
                    nc.vector.tensor_copy(rzb[:], rz32[:])
                    # broadcast 1/Z down this head's 64 partitions
                    nc.tensor.matmul(pzb[off:off + 64, :], lhsT=ones[0:1, 0:64],
                                     rhs=rzb[0:1, :], start=True, stop=True)
                    # unnormalized o^T for this head
                    for mt, (m0, mn) in enumerate(LSPLIT):
                        nc.tensor.matmul(po[off:off + 64, :],
                                         lhsT=vpos[mt][:, h * 64:(h + 1) * 64],
                                         rhs=eT[mt][:], start=(mt == 0),
                                         stop=(mt == 1))
                zb = fe.tile([128, LV], f32, name="zb", tag="zb")
                nc.vector.tensor_copy(zb[:], pzb[:])
                t = fe.tile([128, LV], bf, name=f"ot_{kt2}", tag=f"ot_{kt2}")
                nc.vector.tensor_mul(t[:], po[:], zb[:])
                ot.append(t)

            pt = [fe.tile([128, LV], bf, name=f"pt_{m}", tag=f"pt_{m}")
                  for m in range(KT)]
            ct = [fe.tile([128, LV], bf, name=f"ct_{m}", tag=f"ct_{m}")
                  for m in range(KT)]
            proj(pt, 0, ot, 0, "woT")
            proj(ct, 0, pt, 0, "wc2T", "bc2")
            proj(qlT, cs, ct, 0, "wqlT", "bql")

        # ---- phase 3/4: all-pairs logits, both orientations ----
        # orient 1: rows = l of a, free = m of b  -> max over m -> t2v
        # orient 2: rows = m of b, free = l of a  -> max over l -> v2t
        for orient in range(2):
            for a in range(AL):
                pacc = ps2.tile([1, B], f32, name="p_acc", tag="ps2")
                for lt, (l0, ln) in enumerate(LSPLIT):
                    rm = fe.tile([ln, B], bf, name=f"rm_{lt}", tag=f"rm_{lt}")
                    for bg in range(B // 4):
                        pts = [ps.tile([ln, LV], f32, name="p_lg", tag="ps")
                               for _ in range(4)]
                        for bi in range(4):
                            b = bg * 4 + bi
                            for k in range(KT):
                                if orient == 0:
                                    lhsT = qlT[k][:, a * LV + l0: a * LV + l0 + ln]
                                    rhs = klT[k][:, b * LV:(b + 1) * LV]
                                else:
                                    lhsT = klT[k][:, b * LV + l0: b * LV + l0 + ln]
                                    rhs = qlT[k][:, a * LV:(a + 1) * LV]
                                nc.tensor.matmul(pts[bi][:], lhsT=lhsT, rhs=rhs,
                                                 start=(k == 0), stop=(k == KT - 1))
                        for bi in range(4):
                            b = bg * 4 + bi
                            nc.vector.tensor_reduce(rm[0:ln, b:b + 1], pts[bi][:],
                                                    axis=AX, op=MAX)
                    nc.tensor.matmul(pacc[:], lhsT=ones[0:ln, 0:1], rhs=rm[:],
                                     start=(lt == 0), stop=(lt == 1))
                col = orient * AL * B + a * B
                nc.scalar.mul(out_sb[0:1, col:col + B], pacc[:], 1.0 / LV)

        nc.sync.dma_start(out=d_out[:, :], in_=out_sb[:])


def get_program(reps=1):
    key = ("nc", reps)
    if key not in _CACHE:
        _CACHE[key] = _build_program(reps)
    return _CACHE[key]


def make_in_maps(bef_feat, sent_feat, aft_feat, masks,
                 conv1_w, conv1_b, in_proj_w, out_proj_w, conv2_w, conv2_b,
                 q_w, q_b, k_w, k_b, logit_scale):
    bef_feat = np.asarray(bef_feat, np.float32)
    sent_feat = np.asarray(sent_feat, np.float32)
    aft_feat = np.asarray(aft_feat, np.float32)
    masks = np.asarray(masks, np.float32)

    sent_mean = (sent_feat * masks[:, :, None]).mean(axis=1)       # (B, D)
    txtc = sent_mean @ np.asarray(conv1_w, np.float32)[:, D:].T \
        + np.asarray(conv1_b, np.float32)                          # (B, D)

    aftT = np.ascontiguousarray(
        aft_feat.transpose(2, 0, 1).reshape(D, B * LV)).astype(BF16)

    wmats = {
        "w1aT": np.asarray(conv1_w, np.float32)[:, :D].T,
        "wqT": np.asarray(in_proj_w, np.float32)[0:D, :].T,
        "wkT": np.asarray(in_proj_w, np.float32)[D:2 * D, :].T,
        "wvT": np.asarray(in_proj_w, np.float32)[2 * D:3 * D, :].T,
        "woT": np.asarray(out_proj_w, np.float32).T,
        "wc2T": np.asarray(conv2_w, np.float32).T,
        "wqlT": np.asarray(q_w, np.float32).T,
        "wklT": np.asarray(k_w, np.float32).T,
    }
    wmats = {n: np.ascontiguousarray(v).astype(BF16) for n, v in wmats.items()}
    bvecs = {
        "bc2": np.asarray(conv2_b, np.float32).reshape(1, D).astype(BF16),
        "bql": np.asarray(q_b, np.float32).reshape(1, D).astype(BF16),
        "bkl": np.asarray(k_b, np.float32).reshape(1, D).astype(BF16),
    }

    in_maps = []
    for c in range(NCORES):
        sl = slice(c * AL, (c + 1) * AL)
        befT = np.ascontiguousarray(
            bef_feat[sl].transpose(2, 0, 1).reshape(D, AL * LV)).astype(BF16)
        m = {"befT": befT, "aftT": aftT,
             "txtc": np.ascontiguousarray(
                 txtc[sl].reshape(1, AL * D)).astype(BF16)}
        m.update(wmats)
        m.update(bvecs)
        in_maps.append(m)
    return in_maps


def finish(outs, logit_scale):
    """outs: list of 8 per-core (1, 256) arrays -> scalar loss."""
    t2v = np.zeros((B, B), np.float64)
    v2t = np.zeros((B, B), np.float64)
    for c in range(NCORES):
        o = np.asarray(outs[c], np.float64).reshape(-1)
        for a in range(AL):
            t2v[c * AL + a, :] = o[a * B:(a + 1) * B]
            v2t[c * AL + a, :] = o[AL * B + a * B: AL * B + (a + 1) * B]
    S = 0.5 * (t2v + v2t) * np.exp(np.float64(np.asarray(logit_scale)))

    def ce(m):
        lse = np.log(np.sum(np.exp(m - m.max(axis=1, keepdims=True)), axis=1)) \
            + m.max(axis=1)
        return -np.mean(np.diag(m) - lse)

    loss = 0.5 * (ce(S) + ce(S.T))
    return np.float32(loss)


def kernel(**inputs):
    from concourse.bass_utils import run_bass_kernel_spmd

    nc = get_program()
    in_maps = make_in_maps(**inputs)
    res = run_bass_kernel_spmd(nc, in_maps, core_ids=list(range(NCORES)))
    outs = [res.results[c]["out"] for c in range(NCORES)]
    return finish(outs, inputs["logit_scale"])
